# revision 27
# baseline (speedup 1.0000x reference)
"""Trainium2 Bass kernel for a dense transformer block with top-2-before-causal
attention (B=4, T=1024, C=1024, H=16, n_top=2).

Sharding: 8 cores = 4 batches x 2 T-halves (no collectives). Each core gets its
batch's tokens PERMUTED so its own 512 query tokens come first; K/V cover the
full sequence. Causal masking under the permutation is host-baked into two
staircase tables, so the device program is identical on all cores (SPMD).

Score path is fp32 end-to-end (top-2 selection is knife-edge sensitive).
The top-k threshold is folded into the transposed score matmul via an
augmented contraction row ([K^T; ones] . [Q^T; -kth]) so survivor selection
(s >= kth) is decided by exact fp32 PSUM arithmetic. The causal mask rides a
bf16 identity-matmul accumulation of a -1e30 staircase into the same PSUM.
exp + threshold collapse to one ACT pass + one DVE scalar_tensor_tensor pass
per score tile. Rows with no causal survivor yield denom=0 -> 1/0=inf ->
0*inf=NaN, reproducing the reference's NaN pattern.
"""
import sys
import numpy as np

sys.path.insert(0, "/opt/trn_rl_repo")

B, T, C, H, D = 4, 1024, 1024, 16, 64
TH = T // 2
EPS = 1e-5
NEG = -1.0e30
NT = T // 128     # 8 full-seq tiles
NTO = TH // 128   # 4 own tiles
NCC = C // 128    # 8 channel chunks
NF = 4 * C // 128  # 32 fc tiles

# dtype knobs for non-selection-critical matmuls ("f32" | "f16" | "bf16")
DT_MLP = "f16"
DT_V = "f16"


def _np_dt(s):
    import ml_dtypes
    return {"f32": np.float32, "f16": np.float16, "bf16": ml_dtypes.bfloat16}[s]


def _build(nc):
    """Emit the SPMD program into Bacc `nc`. Identical for every core."""
    import concourse.tile as tile
    from concourse import mybir
    from contextlib import ExitStack

    F32 = mybir.dt.float32
    BF16 = mybir.dt.bfloat16
    MDT = {"f32": F32, "f16": mybir.dt.float16, "bf16": BF16}
    DTM = MDT[DT_MLP]
    DTV = MDT[DT_V]
    AF = mybir.ActivationFunctionType
    ALU = mybir.AluOpType

    def din(name, shape, dtype=F32):
        return nc.dram_tensor(name, shape, dtype, kind="ExternalInput").ap()

    x_nat = din("x_nat", (T, C))            # permuted tokens, natural layout
    xTh = din("xTh", (C, TH))               # x^T own tokens
    wqk = din("wqk", (C, 2 * H * D))        # per-head 128 cols = [K^T | Q^T] dims
    wv = din("wv", (C, C))             # V weights^T, head-grouped cols
    wp = din("wp", (C, C))                  # attn proj^T [c, o]
    wfc = din("wfc", (C, 4 * C), DTM)       # fc^T [c, f]
    wpr = din("wpr", (4 * C, C), DTM)       # proj^T [f, o]
    stairA = din("stairA", (128, 896))
    stairB = din("stairB", (128, 512))
    ident = din("ident", (128, 128))

    out_d = nc.dram_tensor("outT", (C, TH), F32, kind="ExternalOutput").ap()

    with tile.TileContext(nc) as tc, ExitStack() as ctx:
        consts = ctx.enter_context(tc.tile_pool(name="consts", bufs=1))
        pers = ctx.enter_context(tc.tile_pool(name="pers", bufs=1))
        xput = ctx.enter_context(tc.tile_pool(name="xlnTp", bufs=1))

        id_t = consts.tile([128, 128], F32)
        nc.sync.dma_start(id_t[:], ident)
        stA = consts.tile([128, 896], F32)
        nc.sync.dma_start(stA[:], stairA)
        stB = consts.tile([128, 512], F32)
        nc.sync.dma_start(stB[:], stairB)
        ones1 = consts.tile([1, 128], F32)
        nc.vector.memset(ones1[:], 1.0)
        eps_t = consts.tile([128, 1], F32)
        nc.vector.memset(eps_t[:], EPS)

        y_sb = [pers.tile([128, TH], F32, tag=f"y{cc}", name=f"y{cc}") for cc in range(NCC)]
        xlnT = [xput.tile([128, T], F32, tag=f"xlnT{cc}", name=f"xlnT{cc}") for cc in range(NCC)]

        def layernorm_tile(pool, spool, src, tag):
            """LN over the free dim of src [128, C] -> new tile."""
            bns = spool.tile([128, 12], F32, tag=f"bns{tag}")
            nc.vector.bn_stats(bns[:, 0:6], src[:, 0:512])
            nc.vector.bn_stats(bns[:, 6:12], src[:, 512:1024])
            mv = spool.tile([128, 2], F32, tag=f"mv{tag}")
            nc.vector.bn_aggr(mv[:], bns[:])
            sd = spool.tile([128, 1], F32, tag=f"sd{tag}")
            nc.scalar.activation(sd[:], mv[:, 1:2], AF.Sqrt, bias=eps_t[:])
            r = spool.tile([128, 1], F32, tag=f"r{tag}")
            nc.vector.reciprocal(r[:], sd[:])
            nmr = spool.tile([128, 1], F32, tag=f"nmr{tag}")
            nc.vector.scalar_tensor_tensor(nmr[:], mv[:, 0:1], -1.0, r[:],
                                           op0=ALU.mult, op1=ALU.mult)
            dst = pool.tile([128, C], F32, tag=f"ln{tag}")
            nc.scalar.activation(dst[:], src[:], AF.Identity,
                                 bias=nmr[:], scale=r[:])
            return dst

        # ============ Phase 1: LN1 + transpose to xlnT ============
        with tc.tile_pool(name="ln1", bufs=2) as lnp, \
             tc.tile_pool(name="ln1s", bufs=4) as lns, \
             tc.tile_pool(name="ln1p", bufs=4, space="PSUM") as lpp:
            for it in range(NT):
                xt = lnp.tile([128, C], F32, tag="xt")
                nc.sync.dma_start(xt[:], x_nat[it * 128:(it + 1) * 128, :])
                xln = layernorm_tile(lnp, lns, xt, "1")
                for cc in range(NCC):
                    pt = lpp.tile([128, 128], F32, tag="ptr")
                    nc.tensor.transpose(pt[:], xln[:, cc * 128:(cc + 1) * 128],
                                        id_t[:])
                    nc.scalar.copy(xlnT[cc][:, it * 128:(it + 1) * 128], pt[:])

        # ============ Phase 2: attention, two head groups ============
        for g in range(2):
            with tc.tile_pool(name="att", bufs=1) as ap_, \
                 tc.tile_pool(name="attk", bufs=1) as akp:
                ka, qa = {}, {}
                # ---- qkv ----
                with tc.tile_pool(name="attw", bufs=3) as aw, \
                     tc.tile_pool(name="apq", bufs=1, space="PSUM") as apq:
                    for h in range(8):
                        ka[h] = ap_.tile([64, T], F32, tag=f"ka{h}", name=f"ka{h}")
                        qa[h] = ap_.tile([64, TH], F32, tag=f"qa{h}", name=f"qa{h}")
                    for hb in range(4):
                        pq = [[apq.tile([128, TH], F32, tag=f"pq{i}{tf}", name=f"pq{i}{tf}")
                               for tf in range(2)] for i in range(2)]
                        for cc in range(NCC):
                            wt = aw.tile([128, 256], F32, tag="wqkt")
                            hg = g * 8 + hb * 2
                            nc.sync.dma_start(
                                wt[:], wqk[cc * 128:(cc + 1) * 128,
                                           hg * 128:(hg + 2) * 128])
                            for i in range(2):
                                for tf in range(2):
                                    nc.tensor.matmul(
                                        pq[i][tf][:],
                                        wt[:, i * 128:(i + 1) * 128],
                                        xlnT[cc][:, tf * TH:(tf + 1) * TH],
                                        start=(cc == 0), stop=(cc == NCC - 1))
                        for i in range(2):
                            h = hb * 2 + i
                            for tf in range(2):
                                nc.scalar.copy(
                                    ka[h][0:64, tf * TH:(tf + 1) * TH],
                                    pq[i][tf][0:64, :])
                            qtmp = aw.tile([128, TH], F32, tag="qtmp")
                            nc.scalar.copy(qtmp[64:128, :], pq[i][0][64:128, :])
                            nc.sync.dma_start(qa[h][0:64, :], qtmp[64:128, :])

                # ---- V (this group's 8 heads = 512 cols) ----
                v_sb = []
                with tc.tile_pool(name="vps", bufs=2, space="PSUM") as vps:
                    wvt = [akp.tile([128, 512], F32, tag=f"wv{cc}", name=f"wv{cc}")
                           for cc in range(NCC)]
                    for cc in range(NCC):
                        nc.sync.dma_start(
                            wvt[cc][:], wv[cc * 128:(cc + 1) * 128,
                                           g * 512:(g + 1) * 512])
                    for it in range(NT):
                        pv = vps.tile([128, 512], F32, tag="pv")
                        for cc in range(NCC):
                            nc.tensor.matmul(
                                pv[:], xlnT[cc][:, it * 128:(it + 1) * 128],
                                wvt[cc][:], start=(cc == 0),
                                stop=(cc == NCC - 1))
                        vt = ap_.tile([128, 8 * 65], DTV, tag=f"v{it}", name=f"v{it}")
                        vv = vt[:].rearrange("p (h x) -> p h x", h=8)
                        nc.vector.memset(vv[:, :, 64:65], 1.0)
                        nc.scalar.copy(vv[:, :, 0:64], pv[:].rearrange(
                            "p (h x) -> p h x", h=8))
                        v_sb.append(vt)

                # ---- per head: stats, masked scores, PV ----
                with tc.tile_pool(name="attw2", bufs=2) as aw, \
                     tc.tile_pool(name="utp", bufs=1) as utp, \
                     tc.tile_pool(name="psbp", bufs=1) as psp, \
                     tc.tile_pool(name="acp", bufs=1, space="PSUM") as acp, \
                     tc.tile_pool(name="aup", bufs=2, space="PSUM") as aup, \
                     tc.tile_pool(name="ayp", bufs=1, space="PSUM") as ayp:
                    for h in range(8):
                        kth = akp.tile([128, NTO * 8], F32, tag="kth")
                        usb = []
                        for qt in range(NTO):
                            sps = acp.tile([128, T], F32, tag="sps")
                            for kf in range(2):
                                nc.tensor.matmul(
                                    sps[:, kf * TH:(kf + 1) * TH],
                                    qa[h][0:64, qt * 128:(qt + 1) * 128],
                                    ka[h][0:64, kf * TH:(kf + 1) * TH],
                                    start=True, stop=True)
                            ssb = aw.tile([128, T], F32, tag="ssb")
                            nc.scalar.copy(ssb[:], sps[:])
                            nc.vector.max(kth[:, qt * 8:(qt + 1) * 8], ssb[:])
                            nkth = aw.tile([128, 1], F32, tag="nkth")
                            nc.vector.tensor_scalar_mul(
                                nkth[:], kth[:, qt * 8 + 1:qt * 8 + 2], -1.0)
                            ut = utp.tile([128, T], F32, tag=f"ut{qt}",
                                          name=f"ut{qt}")
                            soff = 384 - 128 * qt
                            nc.vector.scalar_tensor_tensor(
                                ut[:, 0:TH], ssb[:, 0:TH], nkth[:],
                                stA[:, soff:soff + TH],
                                op0=ALU.add, op1=ALU.add)
                            nc.vector.scalar_tensor_tensor(
                                ut[:, TH:T], ssb[:, TH:T], nkth[:], stB[:],
                                op0=ALU.add, op1=ALU.add)
                            usb.append(ut)

                        psb = []
                        for kc in range(NT):
                            ups = aup.tile([128, TH], F32, tag="ups")
                            for qt in range(NTO):
                                nc.tensor.transpose(
                                    ups[:, qt * 128:(qt + 1) * 128],
                                    usb[qt][:, kc * 128:(kc + 1) * 128],
                                    id_t[:])
                            esb = aw.tile([128, TH], F32, tag="esb")
                            nc.scalar.activation(esb[:], ups[:], AF.Exp,
                                                 scale=0.125)
                            pt_ = psp.tile([128, TH], DTV, tag=f"pt{kc}", name=f"pt{kc}")
                            nc.vector.scalar_tensor_tensor(
                                pt_[:], ups[:], 0.0, esb[:],
                                op0=ALU.is_ge, op1=ALU.mult)
                            psb.append(pt_)

                        ypv = ayp.tile([65, TH], F32, tag="ypv")
                        for kc in range(NT):
                            vop = v_sb[kc][:].rearrange("p (h x) -> p h x", h=8)
                            nc.tensor.matmul(ypv[:], vop[:, h, :], psb[kc][:],
                                             start=(kc == 0),
                                             stop=(kc == NT - 1))
                        rden = aw.tile([1, TH], F32, tag="rden")
                        nc.vector.reciprocal(rden[:], ypv[64:65, :])
                        rdb = ayp.tile([64, TH], F32, tag="rdb")
                        nc.tensor.matmul(rdb[:], ones1[:, 0:64], rden[:],
                                         start=True, stop=True)
                        rdbs = aw.tile([64, TH], F32, tag="rdbs")
                        nc.scalar.copy(rdbs[:], rdb[:])
                        yn = aw.tile([64, TH], F32, tag="yn")
                        nc.vector.tensor_mul(yn[:], ypv[0:64, :], rdbs[:])
                        hg = g * 8 + h
                        nc.sync.dma_start(
                            y_sb[hg // 2][(hg % 2) * 64:(hg % 2) * 64 + 64, :],
                            yn[:])

        # ============ Phase 3: attn proj + residual ============
        x2p = ctx.enter_context(tc.tile_pool(name="x2p", bufs=1))
        x2T = [x2p.tile([128, TH], F32, tag=f"x2T{cc}", name=f"x2T{cc}")
               for cc in range(NCC)]
        with tc.tile_pool(name="prj", bufs=3) as pw, \
             tc.tile_pool(name="prp", bufs=2, space="PSUM") as prp:
            for ot in range(NCC):
                pp = prp.tile([128, TH], F32, tag="pp")
                for cc in range(NCC):
                    wt = pw.tile([128, 128], F32, tag="wpt")
                    nc.sync.dma_start(
                        wt[:], wp[cc * 128:(cc + 1) * 128,
                                  ot * 128:(ot + 1) * 128])
                    nc.tensor.matmul(pp[:], wt[:], y_sb[cc][:],
                                     start=(cc == 0), stop=(cc == NCC - 1))
                xh = pw.tile([128, TH], F32, tag="xh")
                nc.sync.dma_start(xh[:], xTh[ot * 128:(ot + 1) * 128, :])
                nc.vector.tensor_add(x2T[ot][:], pp[:], xh[:])

        # ============ Phase 4: LN2 (transpose sandwich) ============
        with tc.tile_pool(name="xl2T", bufs=1) as x2put:
            xln2T = [x2put.tile([128, TH], DTM, tag=f"xl2T{cc}", name=f"xl2T{cc}")
                     for cc in range(NCC)]
            with tc.tile_pool(name="ln2", bufs=2) as l2p, \
                 tc.tile_pool(name="ln2s", bufs=4) as l2s, \
                 tc.tile_pool(name="ln2p", bufs=2, space="PSUM") as l2pp:
                for it in range(NTO):
                    x2n = l2p.tile([128, C], F32, tag="x2n")
                    for cc in range(NCC):
                        pt = l2pp.tile([128, 128], F32, tag="pt2")
                        nc.tensor.transpose(
                            pt[:], x2T[cc][:, it * 128:(it + 1) * 128], id_t[:])
                        nc.scalar.copy(x2n[:, cc * 128:(cc + 1) * 128], pt[:])
                    xln2 = layernorm_tile(l2p, l2s, x2n, "2")
                    for cc in range(NCC):
                        pt = l2pp.tile([128, 128], F32, tag="pt3")
                        nc.tensor.transpose(
                            pt[:], xln2[:, cc * 128:(cc + 1) * 128], id_t[:])
                        nc.scalar.copy(
                            xln2T[cc][:, it * 128:(it + 1) * 128], pt[:])

            # ============ Phase 5: MLP ============
            with tc.tile_pool(name="mlp", bufs=1) as mp, \
                 tc.tile_pool(name="mlpw", bufs=3) as mw, \
                 tc.tile_pool(name="mpp", bufs=2, space="PSUM") as mpp:
                h_sb = []
                for ft in range(NF):
                    fp = mpp.tile([128, TH], F32, tag="fp")
                    for cc in range(NCC):
                        wt = mw.tile([128, 128], DTM, tag="wft")
                        nc.sync.dma_start(
                            wt[:], wfc[cc * 128:(cc + 1) * 128,
                                       ft * 128:(ft + 1) * 128])
                        nc.tensor.matmul(fp[:], wt[:], xln2T[cc][:],
                                         start=(cc == 0), stop=(cc == NCC - 1))
                    ht = mp.tile([128, TH], DTM, tag=f"h{ft}", name=f"h{ft}")
                    nc.scalar.activation(ht[:], fp[:], AF.Gelu)
                    h_sb.append(ht)
                for ot in range(NCC):
                    op_ = mpp.tile([128, TH], F32, tag="op")
                    for ft in range(NF):
                        wt = mw.tile([128, 128], DTM, tag="wot")
                        nc.sync.dma_start(
                            wt[:], wpr[ft * 128:(ft + 1) * 128,
                                       ot * 128:(ot + 1) * 128])
                        nc.tensor.matmul(op_[:], wt[:], h_sb[ft][:],
                                         start=(ft == 0), stop=(ft == NF - 1))
                    ot_sb = mw.tile([128, TH], F32, tag="osb")
                    nc.vector.tensor_add(ot_sb[:], op_[:], x2T[ot][:])
                    nc.sync.dma_start(out_d[ot * 128:(ot + 1) * 128, :],
                                      ot_sb[:])

    return nc


_CACHE = {}


def _get_program():
    if "nc" not in _CACHE:
        from concourse import bacc
        nc = bacc.Bacc("TRN2", target_bir_lowering=False, debug=False,
                       num_devices=8)
        _build(nc)
        nc.compile()
        _CACHE["nc"] = nc
    return _CACHE["nc"]


def _host_inputs(x, ln1_w, w_attn, w_attn_proj, ln2_w, w_fc, w_proj):
    """Build the 8 per-core input maps (numpy only)."""
    import ml_dtypes
    BF = ml_dtypes.bfloat16
    DM = _np_dt(DT_MLP)
    DV = _np_dt(DT_V)

    x = np.asarray(x, np.float32)
    ln1_w = np.asarray(ln1_w, np.float32)
    ln2_w = np.asarray(ln2_w, np.float32)
    w_attn = np.asarray(w_attn, np.float32) * ln1_w[None, :]
    w_attn_proj = np.asarray(w_attn_proj, np.float32)
    w_fc = np.asarray(w_fc, np.float32) * ln2_w[None, :]
    w_proj = np.asarray(w_proj, np.float32)

    wq = w_attn[0:C]
    wk = w_attn[C:2 * C]
    wv_ = w_attn[2 * C:3 * C]
    wqk = np.empty((C, 2 * C), np.float32)
    for h in range(H):
        wqk[:, h * 128:h * 128 + 64] = wk[h * 64:(h + 1) * 64].T
        wqk[:, h * 128 + 64:h * 128 + 128] = wq[h * 64:(h + 1) * 64].T
    wv_t = np.ascontiguousarray(wv_.T)
    wp_t = np.ascontiguousarray(w_attn_proj.T)
    wfc_t = np.ascontiguousarray(w_fc.T).astype(DM)
    wpr_t = np.ascontiguousarray(w_proj.T).astype(DM)

    ident = np.eye(128, dtype=np.float32)

    # S-layout staircases: for q-tile qt, slice stairA[:, 384-128qt : 896-128qt]
    # gives mask[q_local_p, k_own] = 0 iff k_own <= qt*128 + p, else NEG.
    jj = np.arange(896)[None, :]
    pp_ = np.arange(128)[:, None]
    stairA = np.where(jj <= pp_ + 384, 0.0, NEG).astype(np.float32)

    in_maps = []
    for core in range(8):
        b, half = core // 2, core % 2
        perm = np.concatenate([np.arange(half * TH, (half + 1) * TH),
                               np.arange((1 - half) * TH, (2 - half) * TH)])
        xp = np.ascontiguousarray(x[b][perm])
        stairB = np.full((128, 512), 0.0 if half == 1 else NEG, np.float32)
        in_maps.append(dict(
            x_nat=xp,
            xTh=np.ascontiguousarray(xp[0:TH].T),
            wqk=wqk, wv=wv_t, wp=wp_t, wfc=wfc_t, wpr=wpr_t,
            stairA=stairA, stairB=stairB,
            ident=ident,
        ))
    return in_maps


def kernel(**inputs):
    from concourse.bass_utils import run_bass_kernel_spmd
    nc = _get_program()
    in_maps = _host_inputs(**inputs)
    res = run_bass_kernel_spmd(nc, in_maps, core_ids=list(range(8)))
    out = np.empty((B, T, C), np.float32)
    for core in range(8):
        b, half = core // 2, core % 2
        outT = res.results[core]["outT"]
        out[b, half * TH:(half + 1) * TH, :] = outT.T
    return out
